# revision 1
# baseline (speedup 1.0000x reference)
"""MinLSTM Trainium2 Bass kernel.

Math (equivalent to the log-space reference, computed in linear space —
stable because the gates are normalized to f+i=1, g>=0, h0=0):

    f_pre = x @ W_f.T + b_f ; i_pre, h_pre likewise
    F = sigmoid(f_pre); I = sigmoid(i_pre); S = F+I
    f = F/S ; i = I/S = 1-f
    g = max(h_pre + 0.5, sigmoid(h_pre))        # exact rewrite of
                                                # where(h>=0, h+0.5, sigmoid(h))
    h_t = f_t * h_{t-1} + i_t * g_t             # tensor_tensor_scan on DVE

Sharding: 8 cores = 4 batches x 2 halves of the D=1024 output channels.
Each core computes gates for its 512 channels over the full sequence via
fp32r matmuls (contraction over all 1024 input channels), then runs the
channel-parallel scan along the free (L) dimension.

Host-side marshalling: x is fed transposed ([D, L] per batch) so the
contraction dim lands on SBUF partitions; weights are fed as W.T slices;
the output is produced as h.T ([E, L]) and transposed back on the host.

Engine layout per core: PE ~166us of fp32r matmuls at 86% occupancy (the
bottleneck; cost-model total 194us, measured ~100-125us/iter steady-state
on HW). Matmuls run w-major per l-chunk so only W_f gates the start; gate
math runs per-chunk on DVE (~97us, incl. the fused
g = max(h_pre+bh+0.5, SH) scalar_tensor_tensor straight from PSUM), ACT
(~61us of sigmoids draining PSUM), and GPSIMD (~76us: S=F+I, t=f*g). DMA
dispatch is spread over SP (weights/outputs), ACT (x loads, split in kb
halves), and SWDGE (biases) so descriptor dispatch never gates the PE.
"""

import numpy as np

B, L, D = 4, 4096, 1024
E = 512           # output channels per core
P = 128           # SBUF partitions
NCHUNK = 512      # matmul l-chunk (= max fp32 moving dim = one PSUM bank)
PAIR = 2 * NCHUNK  # elementwise op granularity
N_L = L // NCHUNK
N_PAIR = L // PAIR
N_K = D // P
N_E = E // P
N_CORES = 8

_prog_cache = {}


def build_program(reps=1):
    key = ("nc", reps)
    if key in _prog_cache:
        return _prog_cache[key]

    import concourse.bass as bass  # noqa: F401
    import concourse.tile as tile
    from concourse import bacc, mybir
    from concourse.mybir import AluOpType as alu

    f32 = mybir.dt.float32
    f32r = mybir.dt.float32r
    sig = mybir.ActivationFunctionType.Sigmoid
    ident = mybir.ActivationFunctionType.Identity

    nc = bacc.Bacc("TRN2", target_bir_lowering=False, debug=False)

    xt = nc.dram_tensor("xt", [D, L], f32r, kind="ExternalInput").ap()
    wts = [
        nc.dram_tensor(n, [D, E], f32r, kind="ExternalInput").ap()
        for n in ("wft", "wit", "wht")
    ]
    biases = {
        n: nc.dram_tensor(n, [E, 1], f32, kind="ExternalInput").ap()
        for n in ("bf", "bi", "bh", "bh2")
    }
    ht = nc.dram_tensor("ht", [E, L], f32, kind="ExternalOutput").ap()

    with tile.TileContext(nc) as tc:
        with (
            tc.tile_pool(name="wpool", bufs=1) as wpool,
            tc.tile_pool(name="bpool", bufs=1) as bpool,
            tc.tile_pool(name="xpool", bufs=2) as xpool,
            tc.tile_pool(name="gpool", bufs=3) as gpool,
            tc.tile_pool(name="hpool", bufs=2) as hpool,
            tc.tile_pool(name="pspool", bufs=8, space="PSUM") as pspool,
        ):
            # Transposed weights, 2 tiles per W: [128(d), (kb 4)(e 512)].
            # One DMA per tile keeps the SP dispatch queue short at startup.
            KH = N_K // 2  # kb per weight tile
            wtile = []  # [w][half] -> tile
            for w in range(3):
                halves = []
                for wh in range(2):
                    t = wpool.tile(
                        [P, KH * E], f32r, tag=f"w{w}_{wh}", name=f"w{w}_{wh}"
                    )
                    tv = t.rearrange("p (kb e) -> p kb e", kb=KH)
                    src = wts[w].rearrange("(kb p) e -> p kb e", kb=N_K)
                    nc.sync.dma_start(
                        out=tv, in_=src[:, wh * KH:(wh + 1) * KH, :]
                    )
                    halves.append(t)
                wtile.append(halves)

            def lhsT(w, kb, eb):
                t = wtile[w][kb // KH]
                base = (kb % KH) * E + eb * P
                return t[:, base:base + P]

            # biases as [128, 4(eb)] tiles, one DMA each (SWDGE: cheap dispatch)
            btile = {}
            for nm in ("bf", "bi", "bh", "bh2"):
                t = bpool.tile([P, N_E], f32, tag=nm, name=f"b_{nm}")
                nc.gpsimd.dma_start(
                    out=t[:],
                    in_=biases[nm].rearrange("(eb p) one -> p (eb one)", eb=N_E),
                )
                btile[nm] = t

            xt3 = xt.rearrange("(kb p) l -> p kb l", kb=N_K)
            h_prev = [None] * N_E

            for rep in range(reps):
              for lc in range(N_L):
                lsl = slice(lc * NCHUNK, (lc + 1) * NCHUNK)
                xtile = xpool.tile(
                    [P, N_K * NCHUNK], f32r, tag="x", name=f"x{rep}_{lc}"
                )
                # x loads dispatch on the ACT HWDGE queue so they don't queue
                # behind weight/output DMAs on SP; two DMAs per chunk (kb
                # halves) so the first accumulation starts after 1 MiB
                xv = xtile.rearrange("p (kb l) -> p kb l", kb=N_K)
                nc.scalar.dma_start(out=xv[:, 0:KH, :], in_=xt3[:, 0:KH, lsl])
                nc.scalar.dma_start(out=xv[:, KH:N_K, :], in_=xt3[:, KH:N_K, lsl])

                # w-major matmul order: only W_f is needed to start a chunk;
                # W_i / W_h stream in behind the first psums. Each psum
                # drains through ACT (sigmoid) right away; the W_h psum is
                # also read by the fused g = max(h_pre + bh2, SH) on DVE.
                gF, gI, gSH, psH = {}, {}, {}, {}
                for w in range(3):
                    for eb in range(N_E):
                        ps = pspool.tile(
                            [P, NCHUNK], f32,
                            tag="psh" if w == 2 else "ps",
                            bufs=3 if w == 2 else 5,
                            name=f"ps{rep}_{lc}_{eb}_{w}",
                        )
                        for kb in range(N_K):
                            nc.tensor.matmul(
                                ps[:],
                                lhsT=lhsT(w, kb, eb),
                                rhs=xtile[:, kb * NCHUNK:(kb + 1) * NCHUNK],
                                start=(kb == 0),
                                stop=(kb == N_K - 1),
                            )
                        beb = slice(eb, eb + 1)
                        if w == 0:
                            gF[eb] = gpool.tile(
                                [P, NCHUNK], f32, tag="F", bufs=5, name=f"F{lc}_{eb}"
                            )
                            nc.scalar.activation(
                                gF[eb][:], ps[:], sig, bias=btile["bf"][:, beb]
                            )
                        elif w == 1:
                            gI[eb] = gpool.tile(
                                [P, NCHUNK], f32, tag="I", bufs=5, name=f"I{lc}_{eb}"
                            )
                            nc.scalar.activation(
                                gI[eb][:], ps[:], sig, bias=btile["bi"][:, beb]
                            )
                        else:
                            gSH[eb] = gpool.tile(
                                [P, NCHUNK], f32, tag="SH", bufs=5, name=f"SH{lc}_{eb}"
                            )
                            nc.scalar.activation(
                                gSH[eb][:], ps[:], sig, bias=btile["bh"][:, beb]
                            )
                            psH[eb] = ps

                for eb in range(N_E):
                    esl = slice(eb * P, (eb + 1) * P)
                    F, I, SH = gF[eb], gI[eb], gSH[eb]

                    S = gpool.tile([P, NCHUNK], f32, tag="S", name=f"S{lc}_{eb}")
                    R = gpool.tile([P, NCHUNK], f32, tag="R", name=f"R{lc}_{eb}")
                    f = gpool.tile([P, NCHUNK], f32, tag="f", name=f"f{lc}_{eb}")
                    g = gpool.tile([P, NCHUNK], f32, tag="g", name=f"g{lc}_{eb}")
                    t_fg = gpool.tile([P, NCHUNK], f32, tag="t", name=f"t{lc}_{eb}")
                    v = gpool.tile([P, NCHUNK], f32, tag="v", name=f"v{lc}_{eb}")

                    nc.gpsimd.tensor_tensor(S[:], F[:], I[:], op=alu.add)
                    nc.vector.reciprocal_approx_fast(R[:], S[:])
                    nc.vector.tensor_tensor(f[:], F[:], R[:], op=alu.mult)
                    # g = max(h_pre + (bh + 0.5), sigmoid(h_pre)) fused from PSUM
                    nc.vector.scalar_tensor_tensor(
                        g[:], psH[eb][:], btile["bh2"][:, eb:eb + 1], SH[:],
                        op0=alu.add, op1=alu.max,
                    )
                    nc.gpsimd.tensor_tensor(t_fg[:], f[:], g[:], op=alu.mult)
                    nc.vector.tensor_tensor(v[:], g[:], t_fg[:], op=alu.subtract)

                    h = hpool.tile([P, NCHUNK], f32, tag=f"h{eb}", name=f"h{lc}_{eb}")
                    initial = 0.0 if lc == 0 else h_prev[eb][:, NCHUNK - 1:NCHUNK]
                    nc.vector.tensor_tensor_scan(
                        h[:], f[:], v[:], initial, op0=alu.mult, op1=alu.add
                    )
                    h_prev[eb] = h

                    nc.sync.dma_start(out=ht[esl, lsl], in_=h[:])

    nc.compile()
    _prog_cache[key] = nc
    return nc


def _in_maps(x, W_f, b_f, W_i, b_i, W_h, b_h):
    x = np.ascontiguousarray(x, dtype=np.float32)
    xts = [np.ascontiguousarray(x[b].T) for b in range(B)]
    maps = []
    for c in range(N_CORES):
        b, half = divmod(c, 2)
        e0 = half * E
        m = {
            "xt": xts[b],
            "wft": np.ascontiguousarray(W_f[e0:e0 + E, :].T, dtype=np.float32),
            "wit": np.ascontiguousarray(W_i[e0:e0 + E, :].T, dtype=np.float32),
            "wht": np.ascontiguousarray(W_h[e0:e0 + E, :].T, dtype=np.float32),
            "bf": np.ascontiguousarray(b_f[e0:e0 + E].reshape(E, 1), dtype=np.float32),
            "bi": np.ascontiguousarray(b_i[e0:e0 + E].reshape(E, 1), dtype=np.float32),
            "bh": np.ascontiguousarray(b_h[e0:e0 + E].reshape(E, 1), dtype=np.float32),
            "bh2": np.ascontiguousarray(
                (b_h[e0:e0 + E] + 0.5).reshape(E, 1), dtype=np.float32
            ),
        }
        maps.append(m)
    return maps


def kernel(x, W_f, b_f, W_i, b_i, W_h, b_h, _trace=False):
    from concourse.bass_utils import run_bass_kernel_spmd

    nc = build_program()
    in_maps = _in_maps(x, W_f, b_f, W_i, b_i, W_h, b_h)
    res = run_bass_kernel_spmd(nc, in_maps, list(range(N_CORES)), trace=_trace)
    _prog_cache["last_result"] = res

    out = np.empty((B, L, D), dtype=np.float32)
    for c in range(N_CORES):
        b, half = divmod(c, 2)
        e0 = half * E
        out[b, :, e0:e0 + E] = res.results[c]["ht"].T
    return out



# revision 3
# speedup vs baseline: 1.3925x; 1.3925x over previous
"""MinLSTM Trainium2 Bass kernel (v2: bf16 matmuls, 1024-wide consumers).

Math (equivalent to the log-space reference, computed in linear space —
stable because the gates are normalized to f+i=1, g>=0, h0=0):

    f_pre = x @ W_f.T + b_f ; i_pre, h_pre likewise
    F = sigmoid(f_pre); I = sigmoid(i_pre); S = F+I
    f = F/S ; i = I/S = 1-f
    g = max(h_pre + 0.5, sigmoid(h_pre))        # exact rewrite of
                                                # where(h>=0, h+0.5, sigmoid(h))
    h_t = f_t * h_{t-1} + i_t * g_t             # tensor_tensor_scan on DVE

Sharding: 8 cores = 4 batches x 2 halves of the D=1024 output channels.
Each core computes gates for its 512 channels over the full sequence,
then runs the channel-parallel scan along the free (L) dimension.

v2 changes vs v1:
  - x and W are fed in bf16 (host-cast): the PE streams bf16 at the same
    1 row/cycle but with FWL on LDWEIGHTS (measured MM stream 175us vs
    182us fp32r for the same 768 matmuls), and x/W DMA bytes halve.
    Gate math stays f32 (measured end-to-end rel err ~2e-3).
  - PSUM tiles are [128, 1024] (2 banks, an L-pair of 512-chunks): the
    two halves are accumulated by back-to-back matmul groups sharing the
    same lhsT, and every consumer (ACT sigmoid, DVE/Pool gate chain,
    scan, output DMA) runs 1024 wide, halving per-op fixed overheads.
  - 4 psum tiles rotate (all 8 banks); per L-pair there are 12 matmul
    groups (3 gates x 4 eb) of 16 matmuls each.

Per-core work: 768 matmuls [K=128 x N=512] = 393K PE cycles ~ 166us at
2.4 GHz — the hard PE roofline for this problem (fp8 fails the accuracy
gate, int8 isn't plumbed). Engine busy (model): ACT ~55us, DVE ~93us,
Pool ~68us, DMA 19 MiB ~60us — all hidden under the PE stream.
"""

import numpy as np
import ml_dtypes

B, L, D = 4, 4096, 1024
E = 512           # output channels per core
P = 128           # SBUF partitions
NCHUNK = 512      # matmul moving dim (= max fp32 moving = one PSUM bank)
PAIR = 2 * NCHUNK  # consumer/op granularity (one 2-bank psum tile)
N_PAIR = L // PAIR
N_K = D // P
N_E = E // P
N_CORES = 8

_prog_cache = {}


def build_program(reps=1):
    key = ("nc", reps)
    if key in _prog_cache:
        return _prog_cache[key]

    import concourse.bass as bass  # noqa: F401
    import concourse.tile as tile
    from concourse import bacc, mybir
    from concourse.mybir import AluOpType as alu

    f32 = mybir.dt.float32
    bf16 = mybir.dt.bfloat16
    sig = mybir.ActivationFunctionType.Sigmoid

    nc = bacc.Bacc("TRN2", target_bir_lowering=False, debug=False)

    xt = nc.dram_tensor("xt", [D, L], bf16, kind="ExternalInput").ap()
    wts = [
        nc.dram_tensor(n, [D, E], bf16, kind="ExternalInput").ap()
        for n in ("wft", "wit", "wht")
    ]
    biases = {
        n: nc.dram_tensor(n, [E, 1], f32, kind="ExternalInput").ap()
        for n in ("bf", "bi", "bh", "bh2")
    }
    ht = nc.dram_tensor("ht", [E, L], f32, kind="ExternalOutput").ap()

    with tile.TileContext(nc) as tc:
        with (
            tc.tile_pool(name="wpool", bufs=1) as wpool,
            tc.tile_pool(name="bpool", bufs=1) as bpool,
            tc.tile_pool(name="xpool", bufs=2) as xpool,
            tc.tile_pool(name="gpool", bufs=2) as gpool,
            tc.tile_pool(name="hpool", bufs=2) as hpool,
            tc.tile_pool(name="pspool", bufs=1, space="PSUM") as pspool,
        ):
            # Transposed weights, 2 tiles per W: [128(d), (kb 4)(e 512)].
            KH = N_K // 2  # kb per weight tile
            wtile = []  # [w][half] -> tile
            for w in range(3):
                halves = []
                for wh in range(2):
                    t = wpool.tile(
                        [P, KH * E], bf16, tag=f"w{w}_{wh}", name=f"w{w}_{wh}"
                    )
                    tv = t.rearrange("p (kb e) -> p kb e", kb=KH)
                    src = wts[w].rearrange("(kb p) e -> p kb e", kb=N_K)
                    nc.sync.dma_start(
                        out=tv, in_=src[:, wh * KH:(wh + 1) * KH, :]
                    )
                    halves.append(t)
                wtile.append(halves)

            def lhsT(w, kb, eb):
                t = wtile[w][kb // KH]
                base = (kb % KH) * E + eb * P
                return t[:, base:base + P]

            # biases as [128, 4(eb)] tiles, one DMA each (SWDGE)
            btile = {}
            for nm in ("bf", "bi", "bh", "bh2"):
                t = bpool.tile([P, N_E], f32, tag=nm, name=f"b_{nm}")
                nc.gpsimd.dma_start(
                    out=t[:],
                    in_=biases[nm].rearrange("(eb p) one -> p (eb one)", eb=N_E),
                )
                btile[nm] = t

            xt3 = xt.rearrange("(kb p) l -> p kb l", kb=N_K)
            h_prev = [None] * N_E

            for rep in range(reps):
              for lp in range(N_PAIR):
                lsl = slice(lp * PAIR, (lp + 1) * PAIR)
                xtile = xpool.tile(
                    [P, N_K * PAIR], bf16, tag="x", name=f"x{rep}_{lp}"
                )
                # x loads on the ACT HWDGE queue; two DMAs per pair (kb
                # halves) so the first accumulation starts after 1 MiB
                xv = xtile.rearrange("p (kb l) -> p kb l", kb=N_K)
                nc.scalar.dma_start(out=xv[:, 0:KH, :], in_=xt3[:, 0:KH, lsl])
                nc.scalar.dma_start(out=xv[:, KH:N_K, :], in_=xt3[:, KH:N_K, lsl])

                # eb-major matmul order (F,I,H per channel block): the
                # consumer chain for eb starts right after its 3 groups,
                # psum tags get >=2 groups of drain slack, and the ACT
                # FIFO order matches psum readiness. Each psum tile is
                # [128, 1024] = 2 banks, two 512-wide accumulation groups
                # back-to-back.
                gF, gI, gSH, psH = {}, {}, {}, {}
                gidx = 0
                for eb in range(N_E):
                    for w in range(3):
                        ps = pspool.tile(
                            [P, PAIR], f32,
                            tag=f"ps{gidx % 4}", bufs=1,
                            name=f"ps{rep}_{lp}_{eb}_{w}",
                        )
                        gidx += 1
                        for half in range(2):
                            osl = slice(half * NCHUNK, (half + 1) * NCHUNK)
                            for kb in range(N_K):
                                nc.tensor.matmul(
                                    ps[:, osl],
                                    lhsT=lhsT(w, kb, eb),
                                    rhs=xtile[
                                        :,
                                        (kb * 2 + half) * NCHUNK:
                                        (kb * 2 + half + 1) * NCHUNK,
                                    ],
                                    start=(kb == 0),
                                    stop=(kb == N_K - 1),
                                )
                        beb = slice(eb, eb + 1)
                        if w == 0:
                            gF[eb] = gpool.tile(
                                [P, PAIR], f32, tag="F", bufs=3, name=f"F{lp}_{eb}"
                            )
                            nc.scalar.activation(
                                gF[eb][:], ps[:], sig, bias=btile["bf"][:, beb]
                            )
                        elif w == 1:
                            gI[eb] = gpool.tile(
                                [P, PAIR], f32, tag="I", bufs=3, name=f"I{lp}_{eb}"
                            )
                            nc.scalar.activation(
                                gI[eb][:], ps[:], sig, bias=btile["bi"][:, beb]
                            )
                        else:
                            gSH[eb] = gpool.tile(
                                [P, PAIR], f32, tag="SH", bufs=3, name=f"SH{lp}_{eb}"
                            )
                            nc.scalar.activation(
                                gSH[eb][:], ps[:], sig, bias=btile["bh"][:, beb]
                            )
                            psH[eb] = ps

                for eb in range(N_E):
                    esl = slice(eb * P, (eb + 1) * P)
                    F, I, SH = gF[eb], gI[eb], gSH[eb]

                    S = gpool.tile([P, PAIR], f32, tag="S", name=f"S{lp}_{eb}")
                    R = gpool.tile([P, PAIR], f32, tag="R", name=f"R{lp}_{eb}")
                    f = gpool.tile([P, PAIR], f32, tag="f", name=f"f{lp}_{eb}")
                    g = gpool.tile([P, PAIR], f32, tag="g", name=f"g{lp}_{eb}")
                    t_fg = gpool.tile([P, PAIR], f32, tag="t", name=f"t{lp}_{eb}")
                    v = gpool.tile([P, PAIR], f32, tag="v", name=f"v{lp}_{eb}")

                    nc.gpsimd.tensor_tensor(S[:], F[:], I[:], op=alu.add)
                    nc.vector.reciprocal_approx_fast(R[:], S[:])
                    nc.vector.tensor_tensor(f[:], F[:], R[:], op=alu.mult)
                    # g = max(h_pre + (bh + 0.5), sigmoid(h_pre)) from PSUM
                    nc.vector.scalar_tensor_tensor(
                        g[:], psH[eb][:], btile["bh2"][:, eb:eb + 1], SH[:],
                        op0=alu.add, op1=alu.max,
                    )
                    nc.gpsimd.tensor_tensor(t_fg[:], f[:], g[:], op=alu.mult)
                    nc.vector.tensor_tensor(v[:], g[:], t_fg[:], op=alu.subtract)

                    h = hpool.tile([P, PAIR], f32, tag=f"h{eb}", name=f"h{lp}_{eb}")
                    initial = 0.0 if lp == 0 else h_prev[eb][:, PAIR - 1:PAIR]
                    nc.vector.tensor_tensor_scan(
                        h[:], f[:], v[:], initial, op0=alu.mult, op1=alu.add
                    )
                    h_prev[eb] = h

                    nc.sync.dma_start(out=ht[esl, lsl], in_=h[:])

    nc.compile()
    _prog_cache[key] = nc
    return nc


def _in_maps(x, W_f, b_f, W_i, b_i, W_h, b_h):
    bf = ml_dtypes.bfloat16
    x = np.ascontiguousarray(x, dtype=np.float32)
    xts = [np.ascontiguousarray(x[b].T).astype(bf) for b in range(B)]
    maps = []
    for c in range(N_CORES):
        b, half = divmod(c, 2)
        e0 = half * E
        m = {
            "xt": xts[b],
            "wft": np.ascontiguousarray(W_f[e0:e0 + E, :].T).astype(bf),
            "wit": np.ascontiguousarray(W_i[e0:e0 + E, :].T).astype(bf),
            "wht": np.ascontiguousarray(W_h[e0:e0 + E, :].T).astype(bf),
            "bf": np.ascontiguousarray(b_f[e0:e0 + E].reshape(E, 1), dtype=np.float32),
            "bi": np.ascontiguousarray(b_i[e0:e0 + E].reshape(E, 1), dtype=np.float32),
            "bh": np.ascontiguousarray(b_h[e0:e0 + E].reshape(E, 1), dtype=np.float32),
            "bh2": np.ascontiguousarray(
                (b_h[e0:e0 + E] + 0.5).reshape(E, 1), dtype=np.float32
            ),
        }
        maps.append(m)
    return maps


def kernel(x, W_f, b_f, W_i, b_i, W_h, b_h, _trace=False):
    from concourse.bass_utils import run_bass_kernel_spmd

    nc = build_program()
    in_maps = _in_maps(x, W_f, b_f, W_i, b_i, W_h, b_h)
    res = run_bass_kernel_spmd(nc, in_maps, list(range(N_CORES)), trace=_trace)
    _prog_cache["last_result"] = res

    out = np.empty((B, L, D), dtype=np.float32)
    for c in range(N_CORES):
        b, half = divmod(c, 2)
        e0 = half * E
        out[b, :, e0:e0 + E] = res.results[c]["ht"].T
    return out


# revision 6
# speedup vs baseline: 1.6389x; 1.1770x over previous
"""MinLSTM Trainium2 Bass kernel (v4: v1 structure, bf16 matmul inputs).

Math (equivalent to the log-space reference, computed in linear space —
stable because the gates are normalized to f+i=1, g>=0, h0=0):

    f_pre = x @ W_f.T + b_f ; i_pre, h_pre likewise
    F = sigmoid(f_pre); I = sigmoid(i_pre); S = F+I
    f = F/S ; i = I/S = 1-f
    g = max(h_pre + 0.5, sigmoid(h_pre))        # exact rewrite of
                                                # where(h>=0, h+0.5, sigmoid(h))
    h_t = f_t * h_{t-1} + i_t * g_t             # tensor_tensor_scan on DVE

Sharding: 8 cores = 4 batches x 2 halves of the D=1024 output channels.
Each core computes gates for its 512 channels over the full sequence via
bf16 matmuls (contraction over all 1024 input channels, fp32 PSUM
accumulate), then runs the channel-parallel scan along the free (L)
dimension. Gate math stays fp32; end-to-end rel err ~2.3e-3 (bf16 x/W
quantization), vs the 2e-2 gate.

Host-side marshalling: x is fed transposed ([D, L] per batch) and cast
to bf16 so the contraction dim lands on SBUF partitions; weights are fed
as W.T bf16 slices; the output is produced as h.T ([E, L]) fp32 and
transposed back on the host.

Engine layout per core: PE is the bottleneck and is at the practical
roofline: 768 matmuls x [K=128, N=512] = 393K cycles = 164us at 2.4GHz;
an MM-stream-only probe measures 228ns/MM (175us) and the full kernel
~178us steady-state (differential, reps=100, paired/interleaved).
bf16 matmuls measure the same 1 row/cycle as fp32r but shave LDWEIGHTS
(FWL) and halve x/W DMA (19 MiB/core total). fp8 fails the accuracy
gate (4e-2 measured in simulation); int8 DoubleRow isn't plumbed in
bass; kb-reuse LDWEIGHTS amortization measured slower (probe: 184 vs
175us). Matmuls run w-major per l-chunk so only W_f gates the start;
gate math runs per-chunk on DVE (~97us, incl. the fused
g = max(h_pre+bh+0.5, SH) scalar_tensor_tensor straight from PSUM), ACT
(~69us of sigmoids draining PSUM), and GPSIMD (~76us: S=F+I, t=f*g) —
all hidden under the PE stream. DMA dispatch is spread over SP
(weights/outputs), ACT (x loads, split in kb halves), and SWDGE
(biases) so descriptor dispatch never gates the PE.
"""

import numpy as np
import ml_dtypes

B, L, D = 4, 4096, 1024
E = 512           # output channels per core
P = 128           # SBUF partitions
NCHUNK = 512      # matmul l-chunk (= max fp32 moving dim = one PSUM bank)
PAIR = 2 * NCHUNK  # elementwise op granularity
N_L = L // NCHUNK
N_PAIR = L // PAIR
N_K = D // P
N_E = E // P
N_CORES = 8

_prog_cache = {}


def build_program(reps=1):
    key = ("nc", reps)
    if key in _prog_cache:
        return _prog_cache[key]

    import concourse.bass as bass  # noqa: F401
    import concourse.tile as tile
    from concourse import bacc, mybir
    from concourse.mybir import AluOpType as alu

    f32 = mybir.dt.float32
    bf16 = mybir.dt.bfloat16
    sig = mybir.ActivationFunctionType.Sigmoid
    ident = mybir.ActivationFunctionType.Identity

    nc = bacc.Bacc("TRN2", target_bir_lowering=False, debug=False)

    xt = nc.dram_tensor("xt", [D, L], bf16, kind="ExternalInput").ap()
    wts = [
        nc.dram_tensor(n, [D, E], bf16, kind="ExternalInput").ap()
        for n in ("wft", "wit", "wht")
    ]
    biases = {
        n: nc.dram_tensor(n, [E, 1], f32, kind="ExternalInput").ap()
        for n in ("bf", "bi", "bh", "bh2")
    }
    ht = nc.dram_tensor("ht", [E, L], f32, kind="ExternalOutput").ap()

    with tile.TileContext(nc) as tc:
        with (
            tc.tile_pool(name="wpool", bufs=1) as wpool,
            tc.tile_pool(name="bpool", bufs=1) as bpool,
            tc.tile_pool(name="xpool", bufs=2) as xpool,
            tc.tile_pool(name="gpool", bufs=3) as gpool,
            tc.tile_pool(name="hpool", bufs=2) as hpool,
            tc.tile_pool(name="pspool", bufs=8, space="PSUM") as pspool,
        ):
            # Transposed weights, 2 tiles per W: [128(d), (kb 4)(e 512)].
            # One DMA per tile keeps the SP dispatch queue short at startup.
            KH = N_K // 2  # kb per weight tile
            wtile = []  # [w][half] -> tile
            for w in range(3):
                halves = []
                for wh in range(2):
                    t = wpool.tile(
                        [P, KH * E], bf16, tag=f"w{w}_{wh}", name=f"w{w}_{wh}"
                    )
                    tv = t.rearrange("p (kb e) -> p kb e", kb=KH)
                    src = wts[w].rearrange("(kb p) e -> p kb e", kb=N_K)
                    nc.sync.dma_start(
                        out=tv, in_=src[:, wh * KH:(wh + 1) * KH, :]
                    )
                    halves.append(t)
                wtile.append(halves)

            def lhsT(w, kb, eb):
                t = wtile[w][kb // KH]
                base = (kb % KH) * E + eb * P
                return t[:, base:base + P]

            # biases as [128, 4(eb)] tiles, one DMA each (SWDGE: cheap dispatch)
            btile = {}
            for nm in ("bf", "bi", "bh", "bh2"):
                t = bpool.tile([P, N_E], f32, tag=nm, name=f"b_{nm}")
                nc.gpsimd.dma_start(
                    out=t[:],
                    in_=biases[nm].rearrange("(eb p) one -> p (eb one)", eb=N_E),
                )
                btile[nm] = t

            xt3 = xt.rearrange("(kb p) l -> p kb l", kb=N_K)
            h_prev = [None] * N_E

            for rep in range(reps):
              for lc in range(N_L):
                lsl = slice(lc * NCHUNK, (lc + 1) * NCHUNK)
                xtile = xpool.tile(
                    [P, N_K * NCHUNK], bf16, tag="x", name=f"x{rep}_{lc}"
                )
                # x loads dispatch on the ACT HWDGE queue so they don't queue
                # behind weight/output DMAs on SP; two DMAs per chunk (kb
                # halves) so the first accumulation starts after 1 MiB
                xv = xtile.rearrange("p (kb l) -> p kb l", kb=N_K)
                nc.scalar.dma_start(out=xv[:, 0:KH, :], in_=xt3[:, 0:KH, lsl])
                nc.scalar.dma_start(out=xv[:, KH:N_K, :], in_=xt3[:, KH:N_K, lsl])

                # w-major matmul order: only W_f is needed to start a chunk;
                # W_i / W_h stream in behind the first psums. Each psum
                # drains through ACT (sigmoid) right away; the W_h psum is
                # also read by the fused g = max(h_pre + bh2, SH) on DVE.
                gF, gI, gSH, psH = {}, {}, {}, {}
                for w in range(3):
                    for eb in range(N_E):
                        ps = pspool.tile(
                            [P, NCHUNK], f32,
                            tag="psh" if w == 2 else "ps",
                            bufs=3 if w == 2 else 5,
                            name=f"ps{rep}_{lc}_{eb}_{w}",
                        )
                        for kb in range(N_K):
                            nc.tensor.matmul(
                                ps[:],
                                lhsT=lhsT(w, kb, eb),
                                rhs=xtile[:, kb * NCHUNK:(kb + 1) * NCHUNK],
                                start=(kb == 0),
                                stop=(kb == N_K - 1),
                            )
                        beb = slice(eb, eb + 1)
                        if w == 0:
                            gF[eb] = gpool.tile(
                                [P, NCHUNK], f32, tag="F", bufs=5, name=f"F{lc}_{eb}"
                            )
                            nc.scalar.activation(
                                gF[eb][:], ps[:], sig, bias=btile["bf"][:, beb]
                            )
                        elif w == 1:
                            gI[eb] = gpool.tile(
                                [P, NCHUNK], f32, tag="I", bufs=5, name=f"I{lc}_{eb}"
                            )
                            nc.scalar.activation(
                                gI[eb][:], ps[:], sig, bias=btile["bi"][:, beb]
                            )
                        else:
                            gSH[eb] = gpool.tile(
                                [P, NCHUNK], f32, tag="SH", bufs=5, name=f"SH{lc}_{eb}"
                            )
                            nc.scalar.activation(
                                gSH[eb][:], ps[:], sig, bias=btile["bh"][:, beb]
                            )
                            psH[eb] = ps

                for eb in range(N_E):
                    esl = slice(eb * P, (eb + 1) * P)
                    F, I, SH = gF[eb], gI[eb], gSH[eb]

                    S = gpool.tile([P, NCHUNK], f32, tag="S", name=f"S{lc}_{eb}")
                    R = gpool.tile([P, NCHUNK], f32, tag="R", name=f"R{lc}_{eb}")
                    f = gpool.tile([P, NCHUNK], f32, tag="f", name=f"f{lc}_{eb}")
                    g = gpool.tile([P, NCHUNK], f32, tag="g", name=f"g{lc}_{eb}")
                    t_fg = gpool.tile([P, NCHUNK], f32, tag="t", name=f"t{lc}_{eb}")
                    v = gpool.tile([P, NCHUNK], f32, tag="v", name=f"v{lc}_{eb}")

                    nc.gpsimd.tensor_tensor(S[:], F[:], I[:], op=alu.add)
                    nc.vector.reciprocal_approx_fast(R[:], S[:])
                    nc.vector.tensor_tensor(f[:], F[:], R[:], op=alu.mult)
                    # g = max(h_pre + (bh + 0.5), sigmoid(h_pre)) fused from PSUM
                    nc.vector.scalar_tensor_tensor(
                        g[:], psH[eb][:], btile["bh2"][:, eb:eb + 1], SH[:],
                        op0=alu.add, op1=alu.max,
                    )
                    nc.gpsimd.tensor_tensor(t_fg[:], f[:], g[:], op=alu.mult)
                    nc.vector.tensor_tensor(v[:], g[:], t_fg[:], op=alu.subtract)

                    h = hpool.tile([P, NCHUNK], f32, tag=f"h{eb}", name=f"h{lc}_{eb}")
                    initial = 0.0 if lc == 0 else h_prev[eb][:, NCHUNK - 1:NCHUNK]
                    nc.vector.tensor_tensor_scan(
                        h[:], f[:], v[:], initial, op0=alu.mult, op1=alu.add
                    )
                    h_prev[eb] = h

                    nc.sync.dma_start(out=ht[esl, lsl], in_=h[:])

    nc.compile()
    _prog_cache[key] = nc
    return nc


def _in_maps(x, W_f, b_f, W_i, b_i, W_h, b_h):
    x = np.ascontiguousarray(x, dtype=np.float32)
    bfd = ml_dtypes.bfloat16
    xts = [np.ascontiguousarray(x[b].T).astype(bfd) for b in range(B)]
    maps = []
    for c in range(N_CORES):
        b, half = divmod(c, 2)
        e0 = half * E
        m = {
            "xt": xts[b],
            "wft": np.ascontiguousarray(W_f[e0:e0 + E, :].T).astype(bfd),
            "wit": np.ascontiguousarray(W_i[e0:e0 + E, :].T).astype(bfd),
            "wht": np.ascontiguousarray(W_h[e0:e0 + E, :].T).astype(bfd),
            "bf": np.ascontiguousarray(b_f[e0:e0 + E].reshape(E, 1), dtype=np.float32),
            "bi": np.ascontiguousarray(b_i[e0:e0 + E].reshape(E, 1), dtype=np.float32),
            "bh": np.ascontiguousarray(b_h[e0:e0 + E].reshape(E, 1), dtype=np.float32),
            "bh2": np.ascontiguousarray(
                (b_h[e0:e0 + E] + 0.5).reshape(E, 1), dtype=np.float32
            ),
        }
        maps.append(m)
    return maps


def kernel(x, W_f, b_f, W_i, b_i, W_h, b_h, _trace=False):
    from concourse.bass_utils import run_bass_kernel_spmd

    nc = build_program()
    in_maps = _in_maps(x, W_f, b_f, W_i, b_i, W_h, b_h)
    res = run_bass_kernel_spmd(nc, in_maps, list(range(N_CORES)), trace=_trace)
    _prog_cache["last_result"] = res

    out = np.empty((B, L, D), dtype=np.float32)
    for c in range(N_CORES):
        b, half = divmod(c, 2)
        e0 = half * E
        out[b, :, e0:e0 + E] = res.results[c]["ht"].T
    return out

